# revision 1
# baseline (speedup 1.0000x reference)
"""Single-head causal attention (B=4, S=2048, D=1024) on 8 TRN2 NeuronCores.

Sharding: core c -> (batch b = c//2, half h = c%2). Each core attends 1024
query rows organized as 4 slots of 256 rows. Slot i of the uniform SPMD
program scans C[i] = (16, 12, 8, 4) key-chunks of 128; the per-core query
rows are assigned so the scan counts cover both cores' causal needs:
  h=0 slots start at q0 = (1792, 1280, 768, 256)   (needs 16/12/8/4 exact)
  h=1 slots start at q0 = (1536, 1024, 512, 0)     (needs 14/10/6/2, padded)
Out-of-causal coverage is zeroed by a host-built multiplicative mask that
also applies the diagonal triangles; at each key-chunk exactly one slot
(the last active one) needs masking.

Everything bf16 on the PE (fp32 PSUM accumulation). Scores are computed
transposed, S^T [key_part, q_free], with a VARIABLE query width per
key-chunk: chunk kc multiplies only against the prefix of slots still
active (width 1024/768/512/256), so causal work at 128-key granularity
costs no extra matmul instructions. exp runs on ScalarE into a packed
et buffer; softmax denominators accumulate on VectorE (acc += et) and
collapse to [1,q] with a single pair of ones-matmuls -- the PE never
burns full-rate rows on [1,N] reductions. Normalization is deferred:
unnormalized ctx flows through the output projection and each [128q,512]
result tile is scaled by 1/denom as a per-partition scalar.

V projection is folded into the output projection (Wvo = Wv @ Wo on the
host), so PV multiplies raw x slices (kept resident in SBUF in bf16) by
e^T. Biases: bq/bk are exactly zero here; bv/bo enter additively as
(bv @ Wo + bo) on the host (attention rows sum to 1).

W matrices are host-reordered into [p, do_block, di, 128] so the first
projection matmul depends on a single 256KB DMA instead of the full 2MB
wall.
"""

import numpy as np
import ml_dtypes

import concourse.bass as bass
import concourse.bacc as bacc
import concourse.mybir as mybir
from concourse.tile import TileContext
from concourse.bass_utils import run_bass_kernel_spmd

B, S, D = 4, 2048, 1024
P = 128
NDC = D // P               # 8 d-chunks
NKC = S // P               # 16 key chunks
NQ = 1024                  # query rows per core
SLOTS = (16, 12, 8, 4)     # key-chunks scanned per 256-q slot
# per-kc active width and packed et offsets
WIDTHS = [256 * (4 - kc // 4) for kc in range(NKC)]
OFF = np.concatenate([[0], np.cumsum(WIDTHS)]).tolist()
ET_TOT = OFF[NKC]          # 10240
F32 = mybir.dt.float32
BF16 = mybir.dt.bfloat16
NPBF16 = ml_dtypes.bfloat16
SCALE = 1.0 / float(np.sqrt(D))

# q-row starts per (h, slot)
Q_STARTS = {0: (1792, 1280, 768, 256), 1: (1536, 1024, 512, 0)}


def _build_program():
    nc = bacc.Bacc("TRN2", target_bir_lowering=False, debug=False)
    xT = nc.declare_dram_parameter("xT", [D, S], BF16, isOutput=False)
    qxT = nc.declare_dram_parameter("qxT", [D, NQ], BF16, isOutput=False)
    xv_d = nc.declare_dram_parameter("xv", [S, D], BF16, isOutput=False)
    wk_d = nc.declare_dram_parameter("Wk", [P, NDC, NDC, P], BF16, isOutput=False)
    wq_d = nc.declare_dram_parameter("Wq", [P, NDC, NDC, P], BF16, isOutput=False)
    wvo_d = nc.declare_dram_parameter("Wvo", [P, NDC, D], BF16, isOutput=False)
    cm_d = nc.declare_dram_parameter("cmask", [NKC, P, 256], BF16, isOutput=False)
    out_d = nc.declare_dram_parameter("o_out", [NQ, D], BF16, isOutput=True)

    with TileContext(nc) as tc:
        with (
            tc.tile_pool(name="persist", bufs=1) as pp,
            tc.tile_pool(name="xta", bufs=2) as xtp,
            tc.tile_pool(name="ctx", bufs=2) as ctp,
            tc.tile_pool(name="ps_s", bufs=2, space="PSUM") as ps_s,
            tc.tile_pool(name="ps_pv", bufs=6, space="PSUM") as ps_pv,
        ):
            # ---- persistent SBUF ----
            kt = [pp.tile([P, S], BF16, name=f"kt{i}") for i in range(NDC)]
            qt = [pp.tile([P, NQ], BF16, name=f"qt{i}") for i in range(NDC)]
            xvr = pp.tile([P, NKC, D], BF16, name="xvr")
            et = pp.tile([P, ET_TOT], BF16, name="et")
            acc = pp.tile([P, NQ], F32, name="acc")
            wvo = pp.tile([P, NDC, D], BF16, name="wvo")
            wk = pp.tile([P, NDC, NDC, P], BF16, name="wk_t")
            wq = pp.tile([P, NDC, NDC, P], BF16, name="wq_t")
            cm_all = pp.tile([P, NKC, 256], BF16, name="cm_all")
            osb_ring = [pp.tile([P, 512], BF16, name=f"osb{i}") for i in range(4)]
            pss_ring = [ps_s.tile([P, 512], F32, name="pss", tag="s") for i in range(2)]
            psv_ring = [ps_pv.tile([P, 512], F32, name="psv") for i in range(6)]
            ones_f = pp.tile([P, 2], F32, name="ones_f")
            nc.vector.memset(ones_f[:], 1.0)
            ones_b = pp.tile([P, 1], BF16, name="ones_b")
            nc.vector.memset(ones_b[:], 1.0)
            acc_h = pp.tile([P, NQ], BF16, name="acc_h")
            d_row = pp.tile([1, NQ], F32, name="d_row")
            d_t = pp.tile([P, 8], F32, name="d_t")
            r_t = pp.tile([P, 8], F32, name="r_t")
            nc.vector.memset(acc[:], 0.0)


            def load_w(dram, w):
                # [p, do, di, 128] -- one DMA per do-block so the first
                # matmul group only waits on 256KB
                for do in range(NDC):
                    nc.sync.dma_start(out=w[:, do], in_=dram[:, do])

            def load_xta(src, col0):
                t = xtp.tile([P, NDC, 512], BF16, name="xta")
                s = src.rearrange("(a p) s -> p a s", p=P)[:, :, col0:col0 + 512]
                for c in range(4):
                    nc.sync.dma_start(
                        out=t[:, 2 * c:2 * c + 2, :], in_=s[:, 2 * c:2 * c + 2, :]
                    )
                return t

            # ---------------- P1/P2: K then Q projections ----------------
            # DMA issue order IS priority (one descriptor stream at ~420
            # GB/s): wk-do0 first so kb0's group 0 can start as soon as the
            # first xta chunks trickle in, then xta0, then the rest of Wk,
            # then xta1 ahead of Wq.
            nc.sync.dma_start(out=wk[:, 0], in_=wk_d[:, 0])
            xta = load_xta(xT, 0)
            for do in range(1, NDC):
                nc.sync.dma_start(out=wk[:, do], in_=wk_d[:, do])
            psn = 0
            for kb in range(4):
                if kb > 0:
                    xta = load_xta(xT, kb * 512)
                if kb == 2:
                    load_w(wq_d, wq)
                for do in range(NDC):
                    ps = pss_ring[psn % 2]; psn += 1
                    for di in range(NDC):
                        nc.tensor.matmul(
                            ps[:],
                            wk[:, do, di, :],
                            xta[:, di, :],
                            start=(di == 0),
                            stop=(di == NDC - 1),
                        )
                    nc.scalar.copy(kt[do][:, kb * 512:(kb + 1) * 512], ps[:])

            # PV x resident + masks + Wvo prefetch under projection compute
            for c in range(NDC):
                nc.sync.dma_start(
                    out=xvr[:, 2 * c:2 * c + 2, :],
                    in_=xv_d.rearrange("(a p) d -> p a d", p=P)[:, 2 * c:2 * c + 2, :],
                )
            for a in range(NDC):
                nc.sync.dma_start(out=wvo[:, a], in_=wvo_d[:, a])
            for cq in range(4):
                nc.sync.dma_start(
                    out=cm_all[:, 4 * cq:4 * cq + 4, :],
                    in_=cm_d.rearrange("a p c -> p a c")[:, 4 * cq:4 * cq + 4, :],
                )

            for qb in range(2):
                xta = load_xta(qxT, qb * 512)
                for do in range(NDC):
                    ps = pss_ring[psn % 2]; psn += 1
                    for di in range(NDC):
                        nc.tensor.matmul(
                            ps[:],
                            wq[:, do, di, :],
                            xta[:, di, :],
                            start=(di == 0),
                            stop=(di == NDC - 1),
                        )
                    nc.scalar.copy(qt[do][:, qb * 512:(qb + 1) * 512], ps[:])

            # ---------------- P3: scores + exp + mask + den-acc ----------
            for kc in range(NKC):
                w = WIDTHS[kc]
                off = OFF[kc]
                parts = [(0, 512), (512, w - 512)] if w > 512 else [(0, w)]
                for (p0, pw) in parts:
                    ps = pss_ring[psn % 2]; psn += 1
                    for di in range(NDC):
                        nc.tensor.matmul(
                            ps[:, 0:pw],
                            kt[di][:, kc * P:(kc + 1) * P],
                            qt[di][:, p0:p0 + pw],
                            start=(di == 0),
                            stop=(di == NDC - 1),
                        )
                    nc.scalar.activation(
                        et[:, off + p0:off + p0 + pw],
                        ps[:, 0:pw],
                        mybir.ActivationFunctionType.Exp,
                        scale=SCALE,
                    )
                nc.vector.tensor_mul(
                    et[:, off + w - 256:off + w], et[:, off + w - 256:off + w],
                    cm_all[:, kc, :],
                )
                nc.vector.tensor_add(
                    acc[:, 0:w], acc[:, 0:w], et[:, off:off + w]
                )

            # bf16 shadow of the denominator accumulator: rounds per-partition
            # partials (err ~0.4%/sqrt(128)) so the den ones-matmuls can run
            # at full bf16 rate instead of quarter-rate fp32
            nc.vector.tensor_copy(acc_h[:], acc[:])

            # ---------------- P4/P5: PV + output projection --------------
            pvn = 0
            osn = 0
            for pr in range(2):
                cmax = SLOTS[2 * pr]       # 16 or 8
                cmin = SLOTS[2 * pr + 1]   # 12 or 4
                qoff = 2 * pr * 256        # et column offset of slot pair
                ctxp = ctp.tile([P, NDC, 512], BF16, name="ctxp")
                for dg in range(2):
                    pss = [psv_ring[(pvn + j) % 6] for j in range(4)]
                    pvn += 4
                    for kc in range(cmax):
                        pw = 512 if kc < cmin else 256
                        stop_a = kc == cmax - 1
                        for j in range(4):
                            dc = dg * 4 + j
                            nc.tensor.matmul(
                                pss[j][:, 0:pw],
                                xvr[:, kc, dc * P:(dc + 1) * P],
                                et[:, OFF[kc] + qoff:OFF[kc] + qoff + pw],
                                start=(kc == 0),
                                stop=stop_a,
                                skip_group_check=True,
                            )
                    for j in range(4):
                        nc.vector.tensor_copy(ctxp[:, dg * 4 + j, :], pss[j][:])
                if pr == 0:
                    # denominators: [1,q] = ones^T @ acc on the PE, then
                    # PE-transpose to [128,8] BEFORE the reciprocal so the
                    # DVE works 128-wide instead of single-partition.
                    # Emitted here (after PV pair 0) so the vector den-acc
                    # chain has drained long before the PE needs anything.
                    onesr = ones_b[:, 0:1]
                    for hh in range(2):
                        psd = ps_s.tile([1, 512], F32, name="psd", tag="s")
                        nc.tensor.matmul(
                            psd[:], onesr, acc_h[:, hh * 512:(hh + 1) * 512],
                            start=True, stop=True,
                        )
                        nc.vector.tensor_copy(
                            d_row[:, hh * 512:(hh + 1) * 512], psd[:]
                        )
                    pst = ps_s.tile([P, 8], F32, name="pst", tag="s")
                    for qs in range(8):
                        nc.tensor.matmul(
                            pst[:, qs:qs + 1],
                            d_row[0:1, qs * P:(qs + 1) * P],
                            ones_f[0:1, 0:1],
                            is_transpose=True,
                            start=True,
                            stop=True,
                        )
                    nc.vector.tensor_copy(d_t[:], pst[:])
                    nc.vector.reciprocal(r_t[:], d_t[:])
                # output projection for the pair's two slots
                for sl in range(2):
                    for qs in range(2):
                        qcol = sl * 256 + qs * P
                        for dh in range(2):
                            pso = psv_ring[pvn % 6]; pvn += 1
                            for dc in range(NDC):
                                nc.tensor.matmul(
                                    pso[:],
                                    ctxp[:, dc, qcol:qcol + P],
                                    wvo[:, dc, dh * 512:(dh + 1) * 512],
                                    start=(dc == 0),
                                    stop=(dc == NDC - 1),
                                )
                            ot = osb_ring[osn % 4]; osn += 1
                            ridx = 4 * pr + sl * 2 + qs
                            if dh == 0:
                                nc.vector.tensor_scalar_mul(
                                    ot[:], pso[:], r_t[:, ridx:ridx + 1]
                                )
                            else:
                                nc.scalar.activation(
                                    ot[:], pso[:],
                                    mybir.ActivationFunctionType.Copy,
                                    scale=r_t[:, ridx:ridx + 1],
                                )
                            nc.sync.dma_start(
                                out=out_d[
                                    (2 * pr + sl) * 256 + qs * P:
                                    (2 * pr + sl) * 256 + (qs + 1) * P,
                                    dh * 512:(dh + 1) * 512,
                                ],
                                in_=ot[:],
                            )
    nc.compile()
    return nc


_PROG = None


def _get_program():
    global _PROG
    if _PROG is None:
        _PROG = _build_program()
    return _PROG


def _make_core_inputs(x, Wq, Wk, Wvo):
    def wre(w):
        # [p, do, di, c] with w[di*128+p, do*128+c]
        return np.ascontiguousarray(
            w.reshape(NDC, P, NDC, P).transpose(1, 2, 0, 3)
        ).astype(NPBF16)

    wq_r = wre(Wq)
    wk_r = wre(Wk)
    wvo_r = np.ascontiguousarray(
        Wvo.reshape(NDC, P, D).transpose(1, 0, 2)
    ).astype(NPBF16)
    qarr = np.arange(256)
    in_maps = []
    for c in range(8):
        b, h = c // 2, c % 2
        q0s = Q_STARTS[h]
        xTb = np.ascontiguousarray(x[b].T).astype(NPBF16)
        qx = np.concatenate([x[b, q0:q0 + 256] for q0 in q0s], axis=0)
        qxT = np.ascontiguousarray(qx.T).astype(NPBF16)
        cm = np.empty((NKC, P, 256), dtype=NPBF16)
        for kc in range(NKC):
            s = 3 - kc // 4
            karr = kc * P + np.arange(P)
            cm[kc] = (karr[:, None] <= (q0s[s] + qarr)[None, :]).astype(NPBF16)
        in_maps.append(
            {
                "xT": xTb,
                "qxT": qxT,
                "xv": x[b].astype(NPBF16),
                "Wq": wq_r,
                "Wk": wk_r,
                "Wvo": wvo_r,
                "cmask": cm,
            }
        )
    return in_maps


def _run(inputs, trace=False, trace_kwargs=None):
    x = np.asarray(inputs["x"], dtype=np.float32)
    Wq = np.asarray(inputs["Wq"], dtype=np.float32)
    Wk = np.asarray(inputs["Wk"], dtype=np.float32)
    Wv = np.asarray(inputs["Wv"], dtype=np.float32)
    Wo = np.asarray(inputs["Wo"], dtype=np.float32)
    bq = np.asarray(inputs["bq"], dtype=np.float32)
    bk = np.asarray(inputs["bk"], dtype=np.float32)
    bv = np.asarray(inputs["bv"], dtype=np.float32)
    bo = np.asarray(inputs["bo"], dtype=np.float32)
    assert not (np.any(bq) or np.any(bk)), "nonzero bq/bk unsupported"

    nc = _get_program()
    in_maps = _make_core_inputs(x, Wq, Wk, Wv @ Wo)
    res = run_bass_kernel_spmd(
        nc, in_maps, list(range(8)), trace=trace, **(trace_kwargs or {})
    )

    out = np.empty((B, S, D), dtype=np.float32)
    for c in range(8):
        b, h = c // 2, c % 2
        o = np.asarray(res.results[c]["o_out"], dtype=np.float32)
        for s, q0 in enumerate(Q_STARTS[h]):
            out[b, q0:q0 + 256] = o[s * 256:(s + 1) * 256]
    out += bv @ Wo + bo                     # exact: attn rows sum to 1
    return out, res


def kernel(**inputs):
    out, _ = _run(inputs)
    return out



# revision 2
# speedup vs baseline: 1.4492x; 1.4492x over previous
"""Single-head causal attention (B=4, S=2048, D=1024) on 8 TRN2 NeuronCores.

Sharding: core c -> (batch b = c//2, half h = c%2). Each core attends 1024
query rows organized as 8 slots of 128 rows. Slot s of the uniform SPMD
program scans C[s] = 16-2s key-chunks of 128; the per-core query rows are
assigned so the scan counts cover both cores' causal needs:
  h=0 slots start at q0 = 128*(15-2s)   (needs 16,14,..,2 exact)
  h=1 slots start at q0 = 128*(14-2s)   (needs 15,13,..,1, padded)
Out-of-causal coverage is zeroed by a host-built multiplicative mask that
also applies the diagonal triangles; at each key-chunk exactly one slot
(the last active one) needs masking.

The K projection is eliminated algebraically: with bq=bk=0,
scores = x Wq Wk^T x^T, so the host precomputes Wqk = Wq @ Wk^T and the
kernel computes t = x_q @ Wqk (same cost as the old Q projection) and
scores directly against the raw x^T input kept resident in SBUF.

Everything bf16 on the PE (fp32 PSUM accumulation). Scores are computed
transposed, S^T [key_part, q_free], with a VARIABLE query width per
key-chunk: chunk kc multiplies only against the prefix of slots still
active (width 128*ceil((16-kc)/2)), so causal work at 128-key granularity
costs no extra matmul instructions. exp runs on ScalarE into a packed
et buffer; softmax denominators accumulate on VectorE (acc += et) and
collapse to [1,q] with a single pair of ones-matmuls -- the PE never
burns full-rate rows on [1,N] reductions. Normalization is deferred:
unnormalized ctx flows through the output projection and each [128q,512]
result tile is scaled by 1/denom as a per-partition scalar.

V projection is folded into the output projection (Wvo = Wv @ Wo on the
host), so PV multiplies raw x slices (kept resident in SBUF in bf16) by
e^T. Biases: bq/bk are exactly zero here; bv/bo enter additively as
(bv @ Wo + bo) on the host (attention rows sum to 1).

Wqk is host-reordered into [p, do_block, di, 128] so the first
projection matmul depends on a single 256KB DMA instead of the full 2MB
wall.
"""

import numpy as np
import ml_dtypes

import concourse.bass as bass
import concourse.bacc as bacc
import concourse.mybir as mybir
from concourse.tile import TileContext
from concourse.bass_utils import run_bass_kernel_spmd

B, S, D = 4, 2048, 1024
P = 128
NDC = D // P               # 8 d-chunks
NKC = S // P               # 16 key chunks
NQ = 1024                  # query rows per core
NSLOT = 8                  # 128-row query slots
# active slot count per key-chunk (prefix of slots), packed et offsets
ACT = [(17 - kc) // 2 for kc in range(NKC)]          # 8,8,7,7,...,1,1
WIDTHS = [P * a for a in ACT]
OFF = np.concatenate([[0], np.cumsum(WIDTHS)]).tolist()
ET_TOT = OFF[NKC]          # 9216
F32 = mybir.dt.float32
BF16 = mybir.dt.bfloat16
NPBF16 = ml_dtypes.bfloat16
SCALE = 1.0 / float(np.sqrt(D))

# q-row starts per (h, slot)
Q_STARTS = {
    0: tuple(P * (15 - 2 * s) for s in range(NSLOT)),
    1: tuple(P * (14 - 2 * s) for s in range(NSLOT)),
}


def _build_program():
    nc = bacc.Bacc("TRN2", target_bir_lowering=False, debug=False)
    xT = nc.declare_dram_parameter("xT", [D, S], BF16, isOutput=False)
    qxT = nc.declare_dram_parameter("qxT", [D, NQ], BF16, isOutput=False)
    xv_d = nc.declare_dram_parameter("xv", [S, D], BF16, isOutput=False)
    wqk_d = nc.declare_dram_parameter("Wqk", [P, NDC, NDC, P], BF16, isOutput=False)
    wvo_d = nc.declare_dram_parameter("Wvo", [P, NDC, D], BF16, isOutput=False)
    cm_d = nc.declare_dram_parameter("cmask", [NKC, P, P], BF16, isOutput=False)
    out_d = nc.declare_dram_parameter("o_out", [NQ, D], BF16, isOutput=True)

    with TileContext(nc) as tc:
        with (
            tc.tile_pool(name="persist", bufs=1) as pp,
            tc.tile_pool(name="xta", bufs=2) as xtp,
            tc.tile_pool(name="ctx", bufs=2) as ctp,
            tc.tile_pool(name="ps_s", bufs=2, space="PSUM") as ps_s,
            tc.tile_pool(name="ps_pv", bufs=6, space="PSUM") as ps_pv,
        ):
            # ---- persistent SBUF ----
            xt_all = pp.tile([P, NDC, S], BF16, name="xt_all")
            qt = [pp.tile([P, NQ], BF16, name=f"qt{i}") for i in range(NDC)]
            xvr = pp.tile([P, NKC, D], BF16, name="xvr")
            et = pp.tile([P, ET_TOT], BF16, name="et")
            acc = pp.tile([P, NQ], F32, name="acc")
            wvo = pp.tile([P, NDC, D], BF16, name="wvo")
            wqk = pp.tile([P, NDC, NDC, P], BF16, name="wqk_t")
            cm_all = pp.tile([P, NKC, P], BF16, name="cm_all")
            osb_ring = [pp.tile([P, 512], BF16, name=f"osb{i}") for i in range(4)]
            pss_ring = [ps_s.tile([P, 512], F32, name="pss", tag="s") for i in range(2)]
            psv_ring = [ps_pv.tile([P, 512], F32, name="psv") for i in range(6)]
            ones_f = pp.tile([P, 2], F32, name="ones_f")
            nc.vector.memset(ones_f[:], 1.0)
            ones_b = pp.tile([P, 1], BF16, name="ones_b")
            nc.vector.memset(ones_b[:], 1.0)
            acc_h = pp.tile([P, NQ], BF16, name="acc_h")
            d_row = pp.tile([1, NQ], F32, name="d_row")
            d_t = pp.tile([P, 8], F32, name="d_t")
            r_t = pp.tile([P, 8], F32, name="r_t")
            nc.vector.memset(acc[:], 0.0)

            def load_qx(src, col0):
                t = xtp.tile([P, NDC, 512], BF16, name="qxa")
                s = src.rearrange("(a p) s -> p a s", p=P)[:, :, col0:col0 + 512]
                for c in range(4):
                    nc.sync.dma_start(
                        out=t[:, 2 * c:2 * c + 2, :], in_=s[:, 2 * c:2 * c + 2, :]
                    )
                return t

            # ---------------- P1: t = x_q @ Wqk projection ----------------
            # DMA issue order IS priority (one descriptor stream at ~420
            # GB/s): wqk-do0 first so the first matmul group only waits on
            # 256KB, then the qb0 query columns, then the rest of Wqk.
            nc.sync.dma_start(out=wqk[:, 0], in_=wqk_d[:, 0])
            qxa = load_qx(qxT, 0)
            for do in range(1, NDC):
                nc.sync.dma_start(out=wqk[:, do], in_=wqk_d[:, do])
            psn = 0
            for qb in range(2):
                if qb > 0:
                    qxa = load_qx(qxT, 512)
                if qb == 1:
                    # x^T resident for scores: issue keyblock-major so the
                    # early key-chunks land first.
                    xts = xT.rearrange("(a p) s -> p a s", p=P)
                    for kb in range(4):
                        for c in range(4):
                            nc.sync.dma_start(
                                out=xt_all[:, 2 * c:2 * c + 2,
                                           kb * 512:(kb + 1) * 512],
                                in_=xts[:, 2 * c:2 * c + 2,
                                        kb * 512:(kb + 1) * 512],
                            )
                for do in range(NDC):
                    ps = pss_ring[psn % 2]; psn += 1
                    for di in range(NDC):
                        nc.tensor.matmul(
                            ps[:],
                            wqk[:, do, di, :],
                            qxa[:, di, :],
                            start=(di == 0),
                            stop=(di == NDC - 1),
                        )
                    nc.scalar.copy(qt[do][:, qb * 512:(qb + 1) * 512], ps[:])

            # masks + PV x resident + Wvo prefetch under projection compute
            for cq in range(4):
                nc.sync.dma_start(
                    out=cm_all[:, 4 * cq:4 * cq + 4, :],
                    in_=cm_d.rearrange("a p c -> p a c")[:, 4 * cq:4 * cq + 4, :],
                )
            for c in range(NDC):
                nc.sync.dma_start(
                    out=xvr[:, 2 * c:2 * c + 2, :],
                    in_=xv_d.rearrange("(a p) d -> p a d", p=P)[:, 2 * c:2 * c + 2, :],
                )
            for a in range(NDC):
                nc.sync.dma_start(out=wvo[:, a], in_=wvo_d[:, a])

            # ---------------- P2: scores + exp + mask + den-acc ----------
            for kc in range(NKC):
                w = WIDTHS[kc]
                off = OFF[kc]
                parts = [(0, 512), (512, w - 512)] if w > 512 else [(0, w)]
                for (p0, pw) in parts:
                    ps = pss_ring[psn % 2]; psn += 1
                    for di in range(NDC):
                        nc.tensor.matmul(
                            ps[:, 0:pw],
                            xt_all[:, di, kc * P:(kc + 1) * P],
                            qt[di][:, p0:p0 + pw],
                            start=(di == 0),
                            stop=(di == NDC - 1),
                        )
                    nc.scalar.activation(
                        et[:, off + p0:off + p0 + pw],
                        ps[:, 0:pw],
                        mybir.ActivationFunctionType.Exp,
                        scale=SCALE,
                    )
                nc.vector.tensor_mul(
                    et[:, off + w - P:off + w], et[:, off + w - P:off + w],
                    cm_all[:, kc, :],
                )
                nc.vector.tensor_add(
                    acc[:, 0:w], acc[:, 0:w], et[:, off:off + w]
                )

            # bf16 shadow of the denominator accumulator: rounds per-partition
            # partials (err ~0.4%/sqrt(128)) so the den ones-matmuls can run
            # at full bf16 rate instead of quarter-rate fp32
            nc.vector.tensor_copy(acc_h[:], acc[:])

            # ---------------- P3/P4: PV + output projection --------------
            # group g covers slots 4g..4g+3 (512 packed et columns at offset
            # 512g within each chunk's active prefix)
            pvn = 0
            osn = 0
            for g in range(2):
                cmax = 16 - 8 * g          # key chunks scanned by the group
                ctxp = ctp.tile([P, NDC, 512], BF16, name="ctxp")
                for dg in range(2):
                    pss = [psv_ring[(pvn + j) % 6] for j in range(4)]
                    pvn += 4
                    for kc in range(cmax):
                        pw = min(WIDTHS[kc] - 512 * g, 512)
                        stop_a = kc == cmax - 1
                        for j in range(4):
                            dc = dg * 4 + j
                            nc.tensor.matmul(
                                pss[j][:, 0:pw],
                                xvr[:, kc, dc * P:(dc + 1) * P],
                                et[:, OFF[kc] + 512 * g:OFF[kc] + 512 * g + pw],
                                start=(kc == 0),
                                stop=stop_a,
                                skip_group_check=True,
                            )
                    for j in range(4):
                        nc.vector.tensor_copy(ctxp[:, dg * 4 + j, :], pss[j][:])
                if g == 0:
                    # denominators: [1,q] = ones^T @ acc on the PE, then
                    # PE-transpose to [128,8] BEFORE the reciprocal so the
                    # DVE works 128-wide instead of single-partition.
                    # Emitted here (after PV group 0) so the vector den-acc
                    # chain has drained long before the PE needs anything.
                    onesr = ones_b[:, 0:1]
                    for hh in range(2):
                        psd = ps_s.tile([1, 512], F32, name="psd", tag="s")
                        nc.tensor.matmul(
                            psd[:], onesr, acc_h[:, hh * 512:(hh + 1) * 512],
                            start=True, stop=True,
                        )
                        nc.vector.tensor_copy(
                            d_row[:, hh * 512:(hh + 1) * 512], psd[:]
                        )
                    pst = ps_s.tile([P, 8], F32, name="pst", tag="s")
                    for qs in range(8):
                        nc.tensor.matmul(
                            pst[:, qs:qs + 1],
                            d_row[0:1, qs * P:(qs + 1) * P],
                            ones_f[0:1, 0:1],
                            is_transpose=True,
                            start=True,
                            stop=True,
                        )
                    nc.vector.tensor_copy(d_t[:], pst[:])
                    nc.vector.reciprocal(r_t[:], d_t[:])
                # output projection for the group's four slots
                for sl in range(4):
                    slot = 4 * g + sl
                    for dh in range(2):
                        pso = psv_ring[pvn % 6]; pvn += 1
                        for dc in range(NDC):
                            nc.tensor.matmul(
                                pso[:],
                                ctxp[:, dc, sl * P:(sl + 1) * P],
                                wvo[:, dc, dh * 512:(dh + 1) * 512],
                                start=(dc == 0),
                                stop=(dc == NDC - 1),
                            )
                        ot = osb_ring[osn % 4]; osn += 1
                        if dh == 0:
                            nc.vector.tensor_scalar_mul(
                                ot[:], pso[:], r_t[:, slot:slot + 1]
                            )
                        else:
                            nc.scalar.activation(
                                ot[:], pso[:],
                                mybir.ActivationFunctionType.Copy,
                                scale=r_t[:, slot:slot + 1],
                            )
                        nc.sync.dma_start(
                            out=out_d[
                                slot * P:(slot + 1) * P,
                                dh * 512:(dh + 1) * 512,
                            ],
                            in_=ot[:],
                        )
    nc.compile()
    return nc


_PROG = None


def _get_program():
    global _PROG
    if _PROG is None:
        _PROG = _build_program()
    return _PROG


def _make_core_inputs(x, Wqk, Wvo):
    def wre(w):
        # [p, do, di, c] with w[di*128+p, do*128+c]
        return np.ascontiguousarray(
            w.reshape(NDC, P, NDC, P).transpose(1, 2, 0, 3)
        ).astype(NPBF16)

    wqk_r = wre(Wqk)
    wvo_r = np.ascontiguousarray(
        Wvo.reshape(NDC, P, D).transpose(1, 0, 2)
    ).astype(NPBF16)
    qarr = np.arange(P)
    in_maps = []
    for c in range(8):
        b, h = c // 2, c % 2
        q0s = Q_STARTS[h]
        xTb = np.ascontiguousarray(x[b].T).astype(NPBF16)
        qx = np.concatenate([x[b, q0:q0 + P] for q0 in q0s], axis=0)
        qxT = np.ascontiguousarray(qx.T).astype(NPBF16)
        cm = np.empty((NKC, P, P), dtype=NPBF16)
        for kc in range(NKC):
            s = ACT[kc] - 1            # last active slot gets the mask
            karr = kc * P + np.arange(P)
            cm[kc] = (karr[:, None] <= (q0s[s] + qarr)[None, :]).astype(NPBF16)
        in_maps.append(
            {
                "xT": xTb,
                "qxT": qxT,
                "xv": x[b].astype(NPBF16),
                "Wqk": wqk_r,
                "Wvo": wvo_r,
                "cmask": cm,
            }
        )
    return in_maps


def _run(inputs, trace=False, trace_kwargs=None):
    x = np.asarray(inputs["x"], dtype=np.float32)
    Wq = np.asarray(inputs["Wq"], dtype=np.float32)
    Wk = np.asarray(inputs["Wk"], dtype=np.float32)
    Wv = np.asarray(inputs["Wv"], dtype=np.float32)
    Wo = np.asarray(inputs["Wo"], dtype=np.float32)
    bq = np.asarray(inputs["bq"], dtype=np.float32)
    bk = np.asarray(inputs["bk"], dtype=np.float32)
    bv = np.asarray(inputs["bv"], dtype=np.float32)
    bo = np.asarray(inputs["bo"], dtype=np.float32)
    assert not (np.any(bq) or np.any(bk)), "nonzero bq/bk unsupported"

    nc = _get_program()
    in_maps = _make_core_inputs(x, Wq @ Wk.T, Wv @ Wo)
    res = run_bass_kernel_spmd(
        nc, in_maps, list(range(8)), trace=trace, **(trace_kwargs or {})
    )

    out = np.empty((B, S, D), dtype=np.float32)
    for c in range(8):
        b, h = c // 2, c % 2
        o = np.asarray(res.results[c]["o_out"], dtype=np.float32)
        for s, q0 in enumerate(Q_STARTS[h]):
            out[b, q0:q0 + P] = o[s * P:(s + 1) * P]
    out += bv @ Wo + bo                     # exact: attn rows sum to 1
    return out, res


def kernel(**inputs):
    out, _ = _run(inputs)
    return out
